# revision 19
# baseline (speedup 1.0000x reference)
"""Trainium2 Bass kernel for EnhancedAttention (B=2, T=2048, D=1024, H=16, DH=64).

Sharding: 8 cores = 2 batches x 4 head-groups (4 heads each). No collectives;
each core computes a partial out-projection (bf16) and the host sums the 4
partials per batch in f32.

v4: head-PAIR attention units with row-tiled S matmuls (the K=64 QK^T
matmuls for the two heads of a pair run concurrently on PE row-groups
0-63 / 64-127), chunk order 0->3 so the densest chunk drains last (keeps
the PE clock warm), host-side pre-arranged input layouts for 4-8KB DMA
descriptors, all x chunks DMA'd upfront, rope muls read proj PSUM
directly, all causal masks on DVE, y output in bf16 spread across 4 DMA
rings, and cross-quadrant DVE writes in the softmax-normalize step
(no DMA hops in the norm path).
"""
import os
import sys

for _p in ("/opt/trn_rl_repo", "/root/.axon_site/_ro/trn_rl_repo"):
    if os.path.isdir(_p) and _p not in sys.path:
        sys.path.append(_p)

import ml_dtypes
import numpy as np

import concourse.bass as bass  # noqa: F401
import concourse.tile as tile
from concourse import bacc, mybir
from concourse.bass_utils import run_bass_kernel_spmd

B, T, D = 2, 2048, 1024
H, DH = 16, 64
HPC = 4  # heads per core
NCORES = 8
ROPE_THETA = 10000.0

F32 = mybir.dt.float32
BF16 = mybir.dt.bfloat16

TCH = 512  # t-chunk (q-chunk) size
TC = T // TCH  # 4
DC = D // 128  # 8 contraction chunks
NKT = T // 128  # 16 k-tiles

# if the cross-quadrant DVE writes misbehave on HW, set False to fall back
# to the DMA-hop norm path
XQUAD_NORM = True


def _rope_tables():
    inv = 1.0 / (ROPE_THETA ** (np.arange(0, DH, 2, dtype=np.float64) / DH))
    f = np.arange(T, dtype=np.float64)[:, None] * inv[None, :]  # [T, 32]
    cos = np.cos(f).T.astype(ml_dtypes.bfloat16)  # [32, T]
    sin = np.sin(f).T.astype(ml_dtypes.bfloat16)
    cs2 = np.ascontiguousarray(np.concatenate([sin, -sin], axis=0))  # [64, T]
    return np.ascontiguousarray(cos), cs2


def _build():
    nc = bacc.Bacc("TRN2", target_bir_lowering=False, debug=False, num_devices=NCORES)
    # host pre-arranged layouts (see kernel() below):
    #   xp:  [TC*128, DC*TCH]  chunk tci rows tci*128..: [p][dc*TCH+t']
    #   wq/wk/wv: [128, DC*HPC*DH]   [p][dc*256+n]
    #   wo:  [128, 2*D]              [p][p2*D+n]
    xp_d = nc.dram_tensor("xp", [TC * 128, DC * TCH], BF16, kind="ExternalInput")
    wq_d = nc.dram_tensor("wq", [128, DC * HPC * DH], BF16, kind="ExternalInput")
    wk_d = nc.dram_tensor("wk", [128, DC * HPC * DH], BF16, kind="ExternalInput")
    wv_d = nc.dram_tensor("wv", [128, DC * HPC * DH], BF16, kind="ExternalInput")
    wo_d = nc.dram_tensor("wo", [128, 2 * D], BF16, kind="ExternalInput")
    y_d = nc.dram_tensor("y", [T, D], BF16, kind="ExternalOutput")

    cs1_np, cs2_np = _rope_tables()
    cs1_d = nc.inline_tensor(cs1_np, "cs1")  # [32, T]
    cs2_d = nc.inline_tensor(cs2_np, "cs2")  # [64, T]

    # causal masks (keep = 1.0): maskt for the kt0 diagonal 128-block,
    # maskd = [zeros(128) | tri(128)] for the extended kt1 diagonal block
    maskt_np = (np.arange(128)[None, :] >= np.arange(128)[:, None])
    maskd_np = (np.arange(256)[None, :] >= np.arange(128)[:, None] + 128)
    maskt_d = nc.inline_tensor(
        np.ascontiguousarray(maskt_np.astype(ml_dtypes.bfloat16)), "maskt"
    )
    maskd_d = nc.inline_tensor(
        np.ascontiguousarray(maskd_np.astype(ml_dtypes.bfloat16)), "maskd"
    )
    # den-broadcast selector rows (both on partition 0 so each K=1 matmul's
    # lhsT is partition-aligned): bcps partitions 0-63 <- denr par1 (odd
    # head), partitions 64-127 <- denr par0 (even head); matches par-swapped ot
    sel_np = np.zeros((1, 2, 128), dtype=np.float32)
    sel_np[0, 1, 0:64] = 1.0
    sel_np[0, 0, 64:128] = 1.0
    sel_d = nc.inline_tensor(sel_np, "selc")

    EXP = mybir.ActivationFunctionType.Exp

    import contextlib
    with tile.TileContext(nc) as tc:
        with (
            contextlib.ExitStack() as _ctx,
            tc.tile_pool(name="sb", bufs=1) as sb,
            tc.tile_pool(name="ropep", bufs=2) as ropep,
            tc.tile_pool(name="ptp", bufs=4) as ptp,
            tc.tile_pool(name="orawp", bufs=2) as orawp,
            tc.tile_pool(name="miscp", bufs=2) as miscp,
            tc.tile_pool(name="ysbp", bufs=3) as ysbp,
        ):
            wq = sb.tile([128, DC, HPC * DH], BF16)
            wk = sb.tile([128, DC, HPC * DH], BF16)
            wv = sb.tile([128, DC, HPC * DH], BF16)
            wo = sb.tile([128, 2, D], BF16)
            cs1 = sb.tile([128, T], BF16)
            cs2 = sb.tile([128, T], BF16)
            qt = [sb.tile([128, T], BF16, tag=f"qt{p}", name=f"qt{p}") for p in range(2)]
            ktt = [sb.tile([128, T], BF16, tag=f"kt{p}", name=f"kt{p}") for p in range(2)]
            maskt = sb.tile([128, 128], BF16, name="maskt")
            maskd = sb.tile([128, 256], BF16, name="maskd")
            vaug = sb.tile([128, NKT // 2, 2, HPC, DH + 1], BF16, name="vaug")
            ot = [sb.tile([128, T], BF16, tag=f"ot{p}", name=f"ot{p}") for p in range(2)]
            sel = sb.tile([1, 2, 128], F32, name="sel")
            xts = [
                sb.tile([128, DC, TCH], BF16, tag=f"xt{tci}", name=f"xt{tci}")
                for tci in range(TC)
            ]

            # ---------------- startup DMAs ----------------
            # sync ring: chunk-0 x first (smallest latency to first proj MM)
            # sync ring carries only chunk-0 x and the rope swap DMAs, so the
            # first attention unit's rope chain is never queued behind bulk
            # loads
            xp_r = xp_d.ap().rearrange("(c p) (d t) -> p c d t", p=128, d=DC)
            nc.sync.dma_start(xts[0][:], xp_r[:, 0])
            # scalar ring: wq, rope tables (+broadcasts), then x chunks 1-3
            nc.scalar.dma_start(wq[:], wq_d.ap().rearrange("p (c n) -> p c n", c=DC))
            nc.scalar.dma_start(cs1[0:32, :], cs1_d.ap())
            nc.scalar.dma_start(cs2[0:64, :], cs2_d.ap())
            for rep in range(1, 4):
                nc.scalar.dma_start(cs1[rep * 32 : (rep + 1) * 32, :], cs1[0:32, :])
            nc.scalar.dma_start(cs2[64:128, :], cs2[0:64, :])
            for tci in range(1, TC):
                nc.scalar.dma_start(xts[tci][:], xp_r[:, tci])
            # gpsimd ring: wk (needed ~3rd piece), masks, wv, wo, sel
            nc.gpsimd.dma_start(wk[:], wk_d.ap().rearrange("p (c n) -> p c n", c=DC))
            nc.gpsimd.dma_start(maskt[:], maskt_d.ap())
            nc.gpsimd.dma_start(maskd[:], maskd_d.ap())

            def load_wv():
                nc.gpsimd.dma_start(
                    wv[:], wv_d.ap().rearrange("p (c n) -> p c n", c=DC)
                )

            def load_wo_sel():
                nc.gpsimd.dma_start(wo[:], wo_d.ap().rearrange("p (a n) -> p a n", a=2))
                nc.gpsimd.dma_start(sel[:], sel_d.ap())

            nc.vector.memset(vaug[:, :, :, :, DH : DH + 1], 1.0)

            # PSUM: pjps(2) + sps(2 tags x 2 banks) + ops(2 tags x 1) = 8 banks
            pjps = _ctx.enter_context(tc.tile_pool(name="pjps", bufs=2, space="PSUM"))
            sps = _ctx.enter_context(tc.tile_pool(name="sps", bufs=1, space="PSUM"))
            ops = _ctx.enter_context(tc.tile_pool(name="ops", bufs=1, space="PSUM"))

            # PE warm-up: covers the startup DMA latency; ramps the PE p-state
            warm = sb.tile([128, TCH], BF16, name="warm")
            nc.vector.memset(warm, 0.0)
            wps = pjps.tile([128, TCH], F32, tag="pj", name="wps")
            for wi in range(8):
                nc.tensor.matmul(
                    wps[:], warm[:, 0:128], warm[:],
                    start=(wi == 0), stop=(wi == 7),
                )

            # ---------------- projection pieces ----------------
            def gen_proj_pieces(tci):
                """8 pieces: [q p0, q p1, k p0, k p1, v0..v3] (each ~8 MMs)."""
                tsl = slice(tci * TCH, (tci + 1) * TCH)
                xt = xts[tci]

                def qk_piece(w_sb, dest, p, pi):
                    def run():
                        ps = pjps.tile([128, TCH], F32, tag="pj", name=f"pj{tci}_{pi}")
                        for dc in range(DC):
                            nc.tensor.matmul(
                                ps[:],
                                w_sb[:, dc, p * 128 : (p + 1) * 128],
                                xt[:, dc, :],
                                start=(dc == 0),
                                stop=(dc == DC - 1),
                            )
                        # rope reads proj PSUM directly
                        t1 = ropep.tile([128, TCH], BF16, tag="t1", name=f"t1_{tci}_{pi}")
                        t2 = ropep.tile([128, TCH], BF16, tag="t2", name=f"t2_{tci}_{pi}")
                        swt = ropep.tile([128, TCH], BF16, tag="swt", name=f"sw{tci}_{pi}")
                        nc.vector.tensor_mul(t1[:], ps[:], cs1[:, tsl])
                        nc.vector.tensor_mul(t2[:], ps[:], cs2[:, tsl])
                        for s in range(4):
                            nc.sync.dma_start(
                                swt[s * 32 : (s + 1) * 32, :],
                                t2[(s ^ 1) * 32 : ((s ^ 1) + 1) * 32, :],
                            )
                        nc.vector.tensor_add(dest[p][:, tsl], t1[:], swt[:])
                    return run

                def v_piece(tt):
                    def run():
                        gt = tci * 4 + tt
                        ps = pjps.tile([128, TCH], F32, tag="pj", name=f"pjv{gt}")
                        for dc in range(DC):
                            nc.tensor.matmul(
                                ps[:, : HPC * DH],
                                xt[:, dc, tt * 128 : (tt + 1) * 128],
                                wv[:, dc, :],
                                start=(dc == 0),
                                stop=(dc == DC - 1),
                            )
                        nc.vector.tensor_copy(
                            vaug[:, gt // 2, gt % 2, :, 0:DH],
                            ps[:, : HPC * DH].rearrange("p (h d) -> p h d", h=HPC),
                        )
                    return run

                pieces = []
                pi = 0
                for w_sb, dest in ((wq, qt), (wk, ktt)):
                    for p in range(2):
                        pieces.append(qk_piece(w_sb, dest, p, pi))
                        pi += 1
                for tt in range(4):
                    pieces.append(v_piece(tt))
                return pieces

            # ---------------- out-projection pieces ----------------
            YRINGS = [nc.sync, nc.gpsimd, nc.sync, nc.gpsimd]

            def gen_outproj_pieces(qc):
                def piece(tt):
                    def run():
                        gtt = qc * 4 + tt
                        ysb = ysbp.tile([128, 2, TCH], BF16, tag="ysb", name=f"ys{gtt}")
                        for ni in range(2):
                            ypsum = pjps.tile(
                                [128, TCH], F32, tag="pj", name=f"y{gtt}_{ni}"
                            )
                            for p2 in range(2):
                                nc.tensor.matmul(
                                    ypsum[:],
                                    ot[p2][:, gtt * 128 : (gtt + 1) * 128],
                                    wo[:, p2, ni * TCH : (ni + 1) * TCH],
                                    start=(p2 == 0),
                                    stop=(p2 == 1),
                                )
                            nc.vector.tensor_copy(ysb[:, ni, :], ypsum[:])
                        YRINGS[tt].dma_start(
                            y_d.ap()[gtt * 128 : (gtt + 1) * 128, :],
                            ysb.rearrange("p a b -> p (a b)"),
                        )
                    return run
                return [piece(tt) for tt in range(4)]

            # ---------------- attention: head-pair units ----------------
            pts = {}     # (qc,hp) -> {g: [pt_par0, pt_par1]}
            opsums = {}  # (qc,hp) -> [psum_par0, psum_par1]

            def emit_s_group(qc, hp, g):
                """Row-tiled S pair + exp + mask for k-tile pair (2g, 2g+1)."""
                kt0 = 2 * g
                off0 = max(0, 128 * kt0 - TCH * qc)
                qsl = slice(qc * TCH, (qc + 1) * TCH)
                spts, ptts = [], []
                for par in (0, 1):
                    spt = sps.tile(
                        [128, 2, TCH], F32, tag=f"s{par}", name=f"s{qc}_{hp}_{g}_{par}"
                    )
                    pt = ptp.tile(
                        [128, 2, TCH], BF16, tag=f"pt{par}", name=f"p{qc}_{hp}_{g}_{par}"
                    )
                    spts.append(spt)
                    ptts.append(pt)
                pts[(qc, hp)][g] = ptts
                for j in (0, 1):
                    kt = kt0 + j
                    for par in (0, 1):
                        nc.tensor.matmul(
                            spts[par][:, j, off0:],
                            ktt[hp][par * 64 : (par + 1) * 64, kt * 128 : (kt + 1) * 128],
                            qt[hp][par * 64 : (par + 1) * 64, qsl][:, off0:],
                            start=True,
                            stop=True,
                        )
                for par in (0, 1):
                    if off0 == 0:
                        nc.scalar.activation(
                            ptts[par].rearrange("p a b -> p (a b)"),
                            spts[par].rearrange("p a b -> p (a b)"),
                            EXP, bias=0.0, scale=0.125,
                        )
                    else:
                        nc.scalar.activation(
                            ptts[par][:, :, off0:], spts[par][:, :, off0:],
                            EXP, bias=0.0, scale=0.125,
                        )
                if kt0 >= 4 * qc:  # diagonal pair
                    for par in (0, 1):
                        nc.vector.tensor_mul(
                            ptts[par][:, 0, off0 : off0 + 128],
                            ptts[par][:, 0, off0 : off0 + 128],
                            maskt[:],
                        )
                        nc.vector.tensor_mul(
                            ptts[par][:, 1, off0 : off0 + 256],
                            ptts[par][:, 1, off0 : off0 + 256],
                            maskd[:],
                        )

            def emit_av_group(qc, hp, g):
                nkt = 4 * qc + 4
                kt0 = 2 * g
                ptts = pts[(qc, hp)].pop(g)
                for par in (0, 1):
                    if g == 0:
                        opsums[(qc, hp)][par] = ops.tile(
                            [128, TCH], F32, tag=f"o{par}", name=f"o{qc}_{hp}_{par}"
                        )
                    h = 2 * hp + par
                    for j in (0, 1):
                        kt = kt0 + j
                        off = max(0, 128 * kt - TCH * qc)
                        nc.tensor.matmul(
                            opsums[(qc, hp)][par][0 : DH + 1, off:],
                            vaug[:, g, j, h, 0 : DH + 1],
                            ptts[par][:, j, off:],
                            start=(kt == 0),
                            stop=(kt == nkt - 1),
                        )

            def emit_evac_norm(qc, hp):
                """Evacuate both heads' AV psums, reciprocal of dens, broadcast
                via a tiny matmul, scale into ot. ot layout par-swapped:
                partitions 0-63 = odd head, 64-127 = even head."""
                qsl = slice(qc * TCH, (qc + 1) * TCH)
                oraw2 = []
                for par in (0, 1):
                    oraw = orawp.tile(
                        [128, TCH], F32, tag=f"or{par}", name=f"or{qc}_{hp}_{par}"
                    )
                    oraw2.append(oraw)
                    nc.vector.tensor_copy(
                        oraw[0 : DH + 1, :], opsums[(qc, hp)][par][0 : DH + 1, :]
                    )
                denr = miscp.tile([1, 2, TCH], F32, tag="denr", name=f"dr{qc}_{hp}")
                denp = miscp.tile([1, 2, TCH], F32, tag="denp", name=f"dp{qc}_{hp}")
                for par in (0, 1):
                    nc.sync.dma_start(
                        denp[:, par, :], oraw2[par][DH : DH + 1, :]
                    )
                nc.vector.reciprocal_approx_fast(
                    out=denr.rearrange("p a b -> p (a b)"),
                    in_=denp.rearrange("p a b -> p (a b)"),
                )
                bcps = ops.tile([128, TCH], F32, tag="o0", name=f"bc{qc}_{hp}")
                nc.tensor.matmul(bcps[:], sel[:, 1, :], denr[:, 1, :], start=True, stop=False)
                nc.tensor.matmul(bcps[:], sel[:, 0, :], denr[:, 0, :], start=False, stop=True)
                nc.vector.tensor_mul(
                    ot[hp][0:64, qsl], oraw2[1][0:64, :], bcps[0:64, :]
                )
                if XQUAD_NORM:
                    nc.vector.tensor_mul(
                        ot[hp][64:128, qsl], oraw2[0][0:64, :], bcps[64:128, :]
                    )
                else:
                    tmpo = miscp.tile([64, TCH], BF16, tag="tmpo", name=f"tp{qc}_{hp}")
                    nc.vector.tensor_mul(
                        tmpo[:], oraw2[0][0:64, :], bcps[64:128, :]
                    )
                    nc.sync.dma_start(ot[hp][64:128, qsl], tmpo[:])

            # ---------------- emission schedule ----------------
            # proj(0) upfront; the two pieces the first attention unit needs
            # (q p0, k p0) go first
            p0 = gen_proj_pieces(0)
            order0 = [p0[0], p0[2], p0[1], load_wv, p0[3], load_wo_sel] + p0[4:]
            for f in order0:
                f()

            p1 = gen_proj_pieces(1)
            p2 = gen_proj_pieces(2)
            p3 = gen_proj_pieces(3)

            op0 = gen_outproj_pieces(0)
            op1 = gen_outproj_pieces(1)
            op2 = gen_outproj_pieces(2)

            units = [(qc, hp) for qc in range(TC) for hp in range(2)]
            # fillers per unit (consumed one per S-group slot; leftovers run
            # at the unit boundary). Invocation order must respect the norms
            # each outproj chunk reads.
            unit_fillers = {
                0: [p1[0], p1[2], p1[1], p1[3]],      # q1/k1 (2 slots + spill)
                1: p1[4:8],                           # v(1)
                2: [p2[0], p2[2], p2[1], p2[3]],
                3: op0 + p2[4:8],                     # outproj(0), v(2)
                4: [p3[0], p3[2], p3[1], p3[3]],
                5: op1,
                6: p3[4:8],                           # v(3)
                7: op2,
            }

            # flat S-slot stream with a fixed AV lag: the AV matmuls for
            # S slot i run at slot i+LAG, so the final unit's AV doesn't
            # bunch up in the drain
            LAG = 2
            slots = []
            for ui, (qc, hp) in enumerate(units):
                for g in range((4 * qc + 4) // 2):
                    slots.append((ui, qc, hp, g))

            def do_av_slot(j):
                ui, qc, hp, g = slots[j]
                emit_av_group(qc, hp, g)
                if g == (4 * qc + 4) // 2 - 1:  # unit's last AV group
                    emit_evac_norm(qc, hp)

            fill = []
            for i, (ui, qc, hp, g) in enumerate(slots):
                if g == 0:
                    fill = list(unit_fillers.get(ui, []))
                    pts[(qc, hp)] = {}
                    opsums[(qc, hp)] = [None, None]
                if fill:
                    fill.pop(0)()
                if i >= LAG:
                    do_av_slot(i - LAG)
                emit_s_group(qc, hp, g)
                if g == (4 * qc + 4) // 2 - 1:
                    for f in fill:
                        f()
                    fill = []

            # drain: the last LAG AV slots, final norm, outproj of last chunk
            for j in range(len(slots) - LAG, len(slots)):
                do_av_slot(j)
            for f in gen_outproj_pieces(3):
                f()
    nc.compile()
    return nc


_NC_CACHE = []


def _get_nc():
    if not _NC_CACHE:
        _NC_CACHE.append(_build())
    return _NC_CACHE[0]


_LAST_RESULTS = []  # stashed BassKernelResults for test harness introspection


def _wo_rows_parswap(Wout_rows):
    """Reorder the 256 Wout rows so each 128-row pair block is [odd-head 64 | even-head 64]."""
    out = np.empty_like(Wout_rows)
    for hp in range(2):
        blk = Wout_rows[hp * 128 : (hp + 1) * 128]
        out[hp * 128 : hp * 128 + 64] = blk[64:128]
        out[hp * 128 + 64 : (hp + 1) * 128] = blk[0:64]
    return out


def kernel(x, Wqkv, Wout, _trace=False, **_trace_kwargs):
    x = np.asarray(x, dtype=np.float32)
    Wqkv = np.asarray(Wqkv, dtype=np.float32)
    Wout = np.asarray(Wout, dtype=np.float32)

    nc = _get_nc()
    bf = ml_dtypes.bfloat16
    in_maps = []
    for c in range(NCORES):
        b, g = divmod(c, HPC)
        cols = slice(g * HPC * DH, (g + 1) * HPC * DH)
        rows = slice(g * HPC * DH, (g + 1) * HPC * DH)
        xT = x[b].T.astype(bf)  # [D, T]
        xp = np.ascontiguousarray(
            xT.reshape(DC, 128, TC, TCH).transpose(2, 1, 0, 3).reshape(TC * 128, DC * TCH)
        )

        def wprep(w):  # [D, 256] -> [128, DC*256]
            return np.ascontiguousarray(
                w.astype(bf).reshape(DC, 128, HPC * DH).transpose(1, 0, 2).reshape(128, -1)
            )

        wo_ = np.ascontiguousarray(
            _wo_rows_parswap(Wout[rows, :]).astype(bf)
            .reshape(2, 128, D).transpose(1, 0, 2).reshape(128, 2 * D)
        )
        in_maps.append(
            {
                "xp": xp,
                "wq": wprep(Wqkv[:, 0:D][:, cols]),
                "wk": wprep(Wqkv[:, D : 2 * D][:, cols]),
                "wv": wprep(Wqkv[:, 2 * D : 3 * D][:, cols]),
                "wo": wo_,
            }
        )

    res = run_bass_kernel_spmd(
        nc, in_maps, core_ids=list(range(NCORES)), trace=_trace, **_trace_kwargs
    )
    _LAST_RESULTS.clear()
    _LAST_RESULTS.append(res)

    out = np.zeros((B, T, D), dtype=np.float32)
    for c in range(NCORES):
        b = c // HPC
        out[b] += res.results[c]["y"].astype(np.float32)
    return out


# revision 26
# speedup vs baseline: 1.0202x; 1.0202x over previous
"""Trainium2 Bass kernel for EnhancedAttention (B=2, T=2048, D=1024, H=16, DH=64).

Sharding: 8 cores = 2 batches x 4 head-groups (4 heads each). No collectives;
each core computes a partial out-projection (bf16) and the host sums the 4
partials per batch in f32.

v4: head-PAIR attention units with row-tiled S matmuls (the K=64 QK^T
matmuls for the two heads of a pair run concurrently on PE row-groups
0-63 / 64-127), chunk order 0->3 so the densest chunk drains last (keeps
the PE clock warm), host-side pre-arranged input layouts for 4-8KB DMA
descriptors, all x chunks DMA'd upfront, rope muls read proj PSUM
directly, all causal masks on DVE, y output in bf16 spread across 4 DMA
rings, and cross-quadrant DVE writes in the softmax-normalize step
(no DMA hops in the norm path).
"""
import os
import sys

for _p in ("/opt/trn_rl_repo", "/root/.axon_site/_ro/trn_rl_repo"):
    if os.path.isdir(_p) and _p not in sys.path:
        sys.path.append(_p)

import ml_dtypes
import numpy as np

import concourse.bass as bass  # noqa: F401
import concourse.tile as tile
from concourse import bacc, mybir
from concourse.bass_utils import run_bass_kernel_spmd

B, T, D = 2, 2048, 1024
H, DH = 16, 64
HPC = 4  # heads per core
NCORES = 8
ROPE_THETA = 10000.0

F32 = mybir.dt.float32
BF16 = mybir.dt.bfloat16

TCH = 512  # t-chunk (q-chunk) size
TC = T // TCH  # 4
DC = D // 128  # 8 contraction chunks
NKT = T // 128  # 16 k-tiles

# if the cross-quadrant DVE writes misbehave on HW, set False to fall back
# to the DMA-hop norm path
XQUAD_NORM = True


def _rope_tables():
    """Full 128-partition rope tables (inlined; avoids SBUF broadcasts)."""
    inv = 1.0 / (ROPE_THETA ** (np.arange(0, DH, 2, dtype=np.float64) / DH))
    f = np.arange(T, dtype=np.float64)[:, None] * inv[None, :]  # [T, 32]
    cos = np.cos(f).T.astype(ml_dtypes.bfloat16)  # [32, T]
    sin = np.sin(f).T.astype(ml_dtypes.bfloat16)
    cs1 = np.ascontiguousarray(np.tile(cos, (4, 1)))  # [128, T]
    cs2 = np.ascontiguousarray(
        np.concatenate([sin, -sin, sin, -sin], axis=0)
    )  # [128, T]
    return cs1, cs2


def _build():
    nc = bacc.Bacc("TRN2", target_bir_lowering=False, debug=False, num_devices=NCORES)
    # host pre-arranged layouts (see kernel() below):
    #   xp:  [TC*128, DC*TCH]  chunk tci rows tci*128..: [p][dc*TCH+t']
    #   wq/wk/wv: [128, DC*HPC*DH]   [p][dc*256+n]
    #   wo:  [128, 2*D]              [p][p2*D+n]
    xp_d = nc.dram_tensor("xp", [TC * 128, DC * TCH], BF16, kind="ExternalInput")
    wq_d = nc.dram_tensor("wq", [128, DC * HPC * DH], BF16, kind="ExternalInput")
    wk_d = nc.dram_tensor("wk", [128, DC * HPC * DH], BF16, kind="ExternalInput")
    wv_d = nc.dram_tensor("wv", [128, DC * HPC * DH], BF16, kind="ExternalInput")
    wo_d = nc.dram_tensor("wo", [128, 2 * D], BF16, kind="ExternalInput")
    y_d = nc.dram_tensor("y", [T, D], BF16, kind="ExternalOutput")

    cs1_np, cs2_np = _rope_tables()
    cs1_d = nc.inline_tensor(cs1_np, "cs1")  # [128, T]
    cs2_d = nc.inline_tensor(cs2_np, "cs2")  # [128, T]

    # causal masks (keep = 1.0): maskt for the kt0 diagonal 128-block,
    # maskd = [zeros(128) | tri(128)] for the extended kt1 diagonal block
    maskt_np = (np.arange(128)[None, :] >= np.arange(128)[:, None])
    maskd_np = (np.arange(256)[None, :] >= np.arange(128)[:, None] + 128)
    maskt_d = nc.inline_tensor(
        np.ascontiguousarray(maskt_np.astype(ml_dtypes.bfloat16)), "maskt"
    )
    maskd_d = nc.inline_tensor(
        np.ascontiguousarray(maskd_np.astype(ml_dtypes.bfloat16)), "maskd"
    )
    # den-broadcast selector rows (both on partition 0 so each K=1 matmul's
    # lhsT is partition-aligned): bcps partitions 0-63 <- denr par1 (odd
    # head), partitions 64-127 <- denr par0 (even head); matches par-swapped ot
    sel_np = np.zeros((1, 2, 128), dtype=np.float32)
    sel_np[0, 1, 0:64] = 1.0
    sel_np[0, 0, 64:128] = 1.0
    sel_d = nc.inline_tensor(sel_np, "selc")

    EXP = mybir.ActivationFunctionType.Exp

    import contextlib
    with tile.TileContext(nc) as tc:
        with (
            contextlib.ExitStack() as _ctx,
            tc.tile_pool(name="sb", bufs=1) as sb,
            tc.tile_pool(name="ropep", bufs=2) as ropep,
            tc.tile_pool(name="ptp", bufs=5) as ptp,
            tc.tile_pool(name="orawp", bufs=2) as orawp,
            tc.tile_pool(name="miscp", bufs=2) as miscp,
            tc.tile_pool(name="ysbp", bufs=3) as ysbp,
        ):
            wq = sb.tile([128, DC, HPC * DH], BF16)
            wk = sb.tile([128, DC, HPC * DH], BF16)
            wv = sb.tile([128, DC, HPC * DH], BF16)
            wo = sb.tile([128, 2, D], BF16)
            cs1 = sb.tile([128, T], BF16)
            cs2 = sb.tile([128, T], BF16)
            qt = [sb.tile([128, T], BF16, tag=f"qt{p}", name=f"qt{p}") for p in range(2)]
            ktt = [sb.tile([128, T], BF16, tag=f"kt{p}", name=f"kt{p}") for p in range(2)]
            maskt = sb.tile([128, 128], BF16, name="maskt")
            maskd = sb.tile([128, 256], BF16, name="maskd")
            vaug = sb.tile([128, NKT // 2, 2, HPC, DH + 1], BF16, name="vaug")
            ot = [sb.tile([128, T], BF16, tag=f"ot{p}", name=f"ot{p}") for p in range(2)]
            sel = sb.tile([1, 2, 128], F32, name="sel")
            xts = [
                sb.tile([128, DC, TCH], BF16, tag=f"xt{tci}", name=f"xt{tci}")
                for tci in range(TC)
            ]

            # ---------------- startup DMAs ----------------
            # sync ring: chunk-0 x first (smallest latency to first proj MM)
            # sync ring carries only chunk-0 x and the rope swap DMAs, so the
            # first attention unit's rope chain is never queued behind bulk
            # loads. None of these issue instructions waits on anything, so no
            # ring engine ever stalls at its queue head during startup.
            xp_r = xp_d.ap().rearrange("(c p) (d t) -> p c d t", p=128, d=DC)
            nc.sync.dma_start(xts[0][:], xp_r[:, 0])
            # scalar ring: proj weights, then x chunks 1-3
            nc.scalar.dma_start(wq[:], wq_d.ap().rearrange("p (c n) -> p c n", c=DC))
            nc.scalar.dma_start(wk[:], wk_d.ap().rearrange("p (c n) -> p c n", c=DC))
            for tci in range(1, TC):
                nc.scalar.dma_start(xts[tci][:], xp_r[:, tci])
            # gpsimd ring: rope tables first (first rope mul ~12us), then
            # masks, wv, wo, sel
            nc.gpsimd.dma_start(cs1[:], cs1_d.ap())
            nc.gpsimd.dma_start(cs2[:], cs2_d.ap())
            nc.gpsimd.dma_start(maskt[:], maskt_d.ap())
            nc.gpsimd.dma_start(maskd[:], maskd_d.ap())
            nc.gpsimd.dma_start(wv[:], wv_d.ap().rearrange("p (c n) -> p c n", c=DC))
            nc.gpsimd.dma_start(wo[:], wo_d.ap().rearrange("p (a n) -> p a n", a=2))
            nc.gpsimd.dma_start(sel[:], sel_d.ap())

            nc.vector.memset(vaug[:, :, :, :, DH : DH + 1], 1.0)

            # PSUM: pjps(2) + sps(2 tags x 2 banks) + ops(2 tags x 1) = 8 banks
            pjps = _ctx.enter_context(tc.tile_pool(name="pjps", bufs=2, space="PSUM"))
            sps = _ctx.enter_context(tc.tile_pool(name="sps", bufs=1, space="PSUM"))
            ops = _ctx.enter_context(tc.tile_pool(name="ops", bufs=1, space="PSUM"))

            # PE warm-up: covers the startup DMA latency; ramps the PE p-state
            warm = sb.tile([128, TCH], BF16, name="warm")
            nc.vector.memset(warm, 0.0)
            wps = pjps.tile([128, TCH], F32, tag="pj", name="wps")
            for wi in range(12):
                nc.tensor.matmul(
                    wps[:], warm[:, 0:128], warm[:],
                    start=(wi == 0), stop=(wi == 11),
                )

            # ---------------- projection pieces ----------------
            def gen_proj_pieces(tci):
                """8 pieces: [q p0, q p1, k p0, k p1, v0..v3] (each ~8 MMs)."""
                tsl = slice(tci * TCH, (tci + 1) * TCH)
                xt = xts[tci]

                def qk_piece(w_sb, dest, p, pi):
                    def run():
                        ps = pjps.tile([128, TCH], F32, tag="pj", name=f"pj{tci}_{pi}")
                        for dc in range(DC):
                            nc.tensor.matmul(
                                ps[:],
                                w_sb[:, dc, p * 128 : (p + 1) * 128],
                                xt[:, dc, :],
                                start=(dc == 0),
                                stop=(dc == DC - 1),
                            )
                        # rope reads proj PSUM directly
                        t1 = ropep.tile([128, TCH], BF16, tag="t1", name=f"t1_{tci}_{pi}")
                        t2 = ropep.tile([128, TCH], BF16, tag="t2", name=f"t2_{tci}_{pi}")
                        swt = ropep.tile([128, TCH], BF16, tag="swt", name=f"sw{tci}_{pi}")
                        nc.vector.tensor_mul(t1[:], ps[:], cs1[:, tsl])
                        nc.vector.tensor_mul(t2[:], ps[:], cs2[:, tsl])
                        for s in range(4):
                            nc.sync.dma_start(
                                swt[s * 32 : (s + 1) * 32, :],
                                t2[(s ^ 1) * 32 : ((s ^ 1) + 1) * 32, :],
                            )
                        nc.vector.tensor_add(dest[p][:, tsl], t1[:], swt[:])
                    return run

                def v_piece(tt):
                    def run():
                        gt = tci * 4 + tt
                        ps = pjps.tile([128, TCH], F32, tag="pj", name=f"pjv{gt}")
                        for dc in range(DC):
                            nc.tensor.matmul(
                                ps[:, : HPC * DH],
                                xt[:, dc, tt * 128 : (tt + 1) * 128],
                                wv[:, dc, :],
                                start=(dc == 0),
                                stop=(dc == DC - 1),
                            )
                        nc.vector.tensor_copy(
                            vaug[:, gt // 2, gt % 2, :, 0:DH],
                            ps[:, : HPC * DH].rearrange("p (h d) -> p h d", h=HPC),
                        )
                    return run

                pieces = []
                pi = 0
                for w_sb, dest in ((wq, qt), (wk, ktt)):
                    for p in range(2):
                        pieces.append(qk_piece(w_sb, dest, p, pi))
                        pi += 1
                for tt in range(4):
                    pieces.append(v_piece(tt))
                return pieces

            # ---------------- out-projection pieces ----------------
            YRINGS = [nc.sync, nc.gpsimd, nc.sync, nc.gpsimd]

            def gen_outproj_pieces(qc):
                def piece(tt):
                    def run():
                        gtt = qc * 4 + tt
                        ysb = ysbp.tile([128, 2, TCH], BF16, tag="ysb", name=f"ys{gtt}")
                        for ni in range(2):
                            ypsum = pjps.tile(
                                [128, TCH], F32, tag="pj", name=f"y{gtt}_{ni}"
                            )
                            for p2 in range(2):
                                nc.tensor.matmul(
                                    ypsum[:],
                                    ot[p2][:, gtt * 128 : (gtt + 1) * 128],
                                    wo[:, p2, ni * TCH : (ni + 1) * TCH],
                                    start=(p2 == 0),
                                    stop=(p2 == 1),
                                )
                            nc.vector.tensor_copy(ysb[:, ni, :], ypsum[:])
                        YRINGS[tt].dma_start(
                            y_d.ap()[gtt * 128 : (gtt + 1) * 128, :],
                            ysb.rearrange("p a b -> p (a b)"),
                        )
                    return run
                return [piece(tt) for tt in range(4)]

            # ---------------- attention: head-pair units ----------------
            pts = {}     # (qc,hp) -> {g: [pt_par0, pt_par1]}
            opsums = {}  # (qc,hp) -> [psum_par0, psum_par1]

            def emit_s_group(qc, hp, g):
                """Row-tiled S pair + exp + mask for k-tile pair (2g, 2g+1)."""
                kt0 = 2 * g
                off0 = max(0, 128 * kt0 - TCH * qc)
                qsl = slice(qc * TCH, (qc + 1) * TCH)
                spts, ptts = [], []
                for par in (0, 1):
                    spt = sps.tile(
                        [128, 2, TCH], F32, tag=f"s{par}", name=f"s{qc}_{hp}_{g}_{par}"
                    )
                    pt = ptp.tile(
                        [128, 2, TCH], BF16, tag=f"pt{par}", name=f"p{qc}_{hp}_{g}_{par}"
                    )
                    spts.append(spt)
                    ptts.append(pt)
                pts[(qc, hp)][g] = ptts
                for j in (0, 1):
                    kt = kt0 + j
                    for par in (0, 1):
                        nc.tensor.matmul(
                            spts[par][:, j, off0:],
                            ktt[hp][par * 64 : (par + 1) * 64, kt * 128 : (kt + 1) * 128],
                            qt[hp][par * 64 : (par + 1) * 64, qsl][:, off0:],
                            start=True,
                            stop=True,
                        )
                for par in (0, 1):
                    if off0 == 0:
                        nc.scalar.activation(
                            ptts[par].rearrange("p a b -> p (a b)"),
                            spts[par].rearrange("p a b -> p (a b)"),
                            EXP, bias=0.0, scale=0.125,
                        )
                    else:
                        nc.scalar.activation(
                            ptts[par][:, :, off0:], spts[par][:, :, off0:],
                            EXP, bias=0.0, scale=0.125,
                        )
                if kt0 >= 4 * qc:  # diagonal pair
                    for par in (0, 1):
                        nc.vector.tensor_mul(
                            ptts[par][:, 0, off0 : off0 + 128],
                            ptts[par][:, 0, off0 : off0 + 128],
                            maskt[:],
                        )
                        nc.vector.tensor_mul(
                            ptts[par][:, 1, off0 : off0 + 256],
                            ptts[par][:, 1, off0 : off0 + 256],
                            maskd[:],
                        )

            def emit_av_group(qc, hp, g):
                nkt = 4 * qc + 4
                kt0 = 2 * g
                ptts = pts[(qc, hp)].pop(g)
                for par in (0, 1):
                    if g == 0:
                        opsums[(qc, hp)][par] = ops.tile(
                            [128, TCH], F32, tag=f"o{par}", name=f"o{qc}_{hp}_{par}"
                        )
                    h = 2 * hp + par
                    for j in (0, 1):
                        kt = kt0 + j
                        off = max(0, 128 * kt - TCH * qc)
                        nc.tensor.matmul(
                            opsums[(qc, hp)][par][0 : DH + 1, off:],
                            vaug[:, g, j, h, 0 : DH + 1],
                            ptts[par][:, j, off:],
                            start=(kt == 0),
                            stop=(kt == nkt - 1),
                        )

            def emit_evac_norm(qc, hp):
                """Evacuate both heads' AV psums, reciprocal of dens, broadcast
                via a tiny matmul, scale into ot. ot layout par-swapped:
                partitions 0-63 = odd head, 64-127 = even head."""
                qsl = slice(qc * TCH, (qc + 1) * TCH)
                oraw2 = []
                for par in (0, 1):
                    oraw = orawp.tile(
                        [128, TCH], F32, tag=f"or{par}", name=f"or{qc}_{hp}_{par}"
                    )
                    oraw2.append(oraw)
                    nc.vector.tensor_copy(
                        oraw[0 : DH + 1, :], opsums[(qc, hp)][par][0 : DH + 1, :]
                    )
                denr = miscp.tile([1, 2, TCH], F32, tag="denr", name=f"dr{qc}_{hp}")
                denp = miscp.tile([1, 2, TCH], F32, tag="denp", name=f"dp{qc}_{hp}")
                for par in (0, 1):
                    nc.sync.dma_start(
                        denp[:, par, :], oraw2[par][DH : DH + 1, :]
                    )
                nc.vector.reciprocal_approx_fast(
                    out=denr.rearrange("p a b -> p (a b)"),
                    in_=denp.rearrange("p a b -> p (a b)"),
                )
                bcps = ops.tile([128, TCH], F32, tag="o0", name=f"bc{qc}_{hp}")
                nc.tensor.matmul(bcps[:], sel[:, 1, :], denr[:, 1, :], start=True, stop=False)
                nc.tensor.matmul(bcps[:], sel[:, 0, :], denr[:, 0, :], start=False, stop=True)
                nc.vector.tensor_mul(
                    ot[hp][0:64, qsl], oraw2[1][0:64, :], bcps[0:64, :]
                )
                if XQUAD_NORM:
                    nc.vector.tensor_mul(
                        ot[hp][64:128, qsl], oraw2[0][0:64, :], bcps[64:128, :]
                    )
                else:
                    tmpo = miscp.tile([64, TCH], BF16, tag="tmpo", name=f"tp{qc}_{hp}")
                    nc.vector.tensor_mul(
                        tmpo[:], oraw2[0][0:64, :], bcps[64:128, :]
                    )
                    nc.sync.dma_start(ot[hp][64:128, qsl], tmpo[:])

            # ---------------- emission schedule ----------------
            # proj(0) upfront; the two pieces the first attention unit needs
            # (q p0, k p0) go first
            p0 = gen_proj_pieces(0)
            for f in [p0[0], p0[2], p0[1], p0[3]] + p0[4:]:
                f()

            p1 = gen_proj_pieces(1)
            p2 = gen_proj_pieces(2)
            p3 = gen_proj_pieces(3)

            op0 = gen_outproj_pieces(0)
            op1 = gen_outproj_pieces(1)
            op2 = gen_outproj_pieces(2)

            units = [(qc, hp) for qc in range(TC) for hp in range(2)]
            # fillers per unit (consumed one per S-group slot; leftovers run
            # at the unit boundary). Invocation order must respect the norms
            # each outproj chunk reads.
            unit_fillers = {
                0: [p1[0], p1[2], p1[1], p1[3]],      # q1/k1 (2 slots + spill)
                1: p1[4:8],                           # v(1)
                2: [p2[0], p2[2], p2[1], p2[3]],
                3: op0 + p2[4:8],                     # outproj(0), v(2)
                4: [p3[0], p3[2], p3[1], p3[3]],
                5: op1,
                6: p3[4:8],                           # v(3)
                7: op2,
            }

            # flat S-slot stream with a fixed AV lag: the AV matmuls for
            # S slot i run at slot i+LAG, so the final unit's AV doesn't
            # bunch up in the drain
            LAG = 3
            slots = []
            for ui, (qc, hp) in enumerate(units):
                for g in range((4 * qc + 4) // 2):
                    slots.append((ui, qc, hp, g))

            def do_av_slot(j):
                ui, qc, hp, g = slots[j]
                emit_av_group(qc, hp, g)
                if g == (4 * qc + 4) // 2 - 1:  # unit's last AV group
                    emit_evac_norm(qc, hp)

            fill = []
            for i, (ui, qc, hp, g) in enumerate(slots):
                if g == 0:
                    fill = list(unit_fillers.get(ui, []))
                    pts[(qc, hp)] = {}
                    opsums[(qc, hp)] = [None, None]
                if fill:
                    fill.pop(0)()
                if i >= LAG:
                    do_av_slot(i - LAG)
                emit_s_group(qc, hp, g)
                if g == (4 * qc + 4) // 2 - 1:
                    for f in fill:
                        f()
                    fill = []

            # drain: the last LAG AV slots, final norm, outproj of last chunk
            for j in range(len(slots) - LAG, len(slots)):
                do_av_slot(j)
            for f in gen_outproj_pieces(3):
                f()
    nc.compile()
    return nc


_NC_CACHE = []


def _get_nc():
    if not _NC_CACHE:
        _NC_CACHE.append(_build())
    return _NC_CACHE[0]


_LAST_RESULTS = []  # stashed BassKernelResults for test harness introspection


def _wo_rows_parswap(Wout_rows):
    """Reorder the 256 Wout rows so each 128-row pair block is [odd-head 64 | even-head 64]."""
    out = np.empty_like(Wout_rows)
    for hp in range(2):
        blk = Wout_rows[hp * 128 : (hp + 1) * 128]
        out[hp * 128 : hp * 128 + 64] = blk[64:128]
        out[hp * 128 + 64 : (hp + 1) * 128] = blk[0:64]
    return out


def kernel(x, Wqkv, Wout, _trace=False, **_trace_kwargs):
    x = np.asarray(x, dtype=np.float32)
    Wqkv = np.asarray(Wqkv, dtype=np.float32)
    Wout = np.asarray(Wout, dtype=np.float32)

    nc = _get_nc()
    bf = ml_dtypes.bfloat16
    in_maps = []
    for c in range(NCORES):
        b, g = divmod(c, HPC)
        cols = slice(g * HPC * DH, (g + 1) * HPC * DH)
        rows = slice(g * HPC * DH, (g + 1) * HPC * DH)
        xT = x[b].T.astype(bf)  # [D, T]
        xp = np.ascontiguousarray(
            xT.reshape(DC, 128, TC, TCH).transpose(2, 1, 0, 3).reshape(TC * 128, DC * TCH)
        )

        def wprep(w):  # [D, 256] -> [128, DC*256]
            return np.ascontiguousarray(
                w.astype(bf).reshape(DC, 128, HPC * DH).transpose(1, 0, 2).reshape(128, -1)
            )

        wo_ = np.ascontiguousarray(
            _wo_rows_parswap(Wout[rows, :]).astype(bf)
            .reshape(2, 128, D).transpose(1, 0, 2).reshape(128, 2 * D)
        )
        in_maps.append(
            {
                "xp": xp,
                "wq": wprep(Wqkv[:, 0:D][:, cols]),
                "wk": wprep(Wqkv[:, D : 2 * D][:, cols]),
                "wv": wprep(Wqkv[:, 2 * D : 3 * D][:, cols]),
                "wo": wo_,
            }
        )

    res = run_bass_kernel_spmd(
        nc, in_maps, core_ids=list(range(NCORES)), trace=_trace, **_trace_kwargs
    )
    _LAST_RESULTS.clear()
    _LAST_RESULTS.append(res)

    out = np.zeros((B, T, D), dtype=np.float32)
    for c in range(NCORES):
        b = c // HPC
        out[b] += res.results[c]["y"].astype(np.float32)
    return out


# revision 29
# speedup vs baseline: 1.0466x; 1.0258x over previous
"""Trainium2 Bass kernel for EnhancedAttention (B=2, T=2048, D=1024, H=16, DH=64).

Sharding: 8 cores = 2 batches x 4 head-groups (4 heads each). No collectives;
each core computes a partial out-projection (bf16) and the host sums the 4
partials per batch in f32.

v4: head-PAIR attention units with row-tiled S matmuls (the K=64 QK^T
matmuls for the two heads of a pair run concurrently on PE row-groups
0-63 / 64-127), chunk order 0->3 so the densest chunk drains last (keeps
the PE clock warm), host-side pre-arranged input layouts for 4-8KB DMA
descriptors, all x chunks DMA'd upfront, rope muls read proj PSUM
directly, all causal masks on DVE, y output in bf16 spread across 4 DMA
rings, and cross-quadrant DVE writes in the softmax-normalize step
(no DMA hops in the norm path).
"""
import os
import sys

for _p in ("/opt/trn_rl_repo", "/root/.axon_site/_ro/trn_rl_repo"):
    if os.path.isdir(_p) and _p not in sys.path:
        sys.path.append(_p)

import ml_dtypes
import numpy as np

import concourse.bass as bass  # noqa: F401
import concourse.tile as tile
from concourse import bacc, mybir
from concourse.bass_utils import run_bass_kernel_spmd

B, T, D = 2, 2048, 1024
H, DH = 16, 64
HPC = 4  # heads per core
NCORES = 8
ROPE_THETA = 10000.0

F32 = mybir.dt.float32
BF16 = mybir.dt.bfloat16

TCH = 512  # t-chunk (q-chunk) size
TC = T // TCH  # 4
DC = D // 128  # 8 contraction chunks
NKT = T // 128  # 16 k-tiles

# if the cross-quadrant DVE writes misbehave on HW, set False to fall back
# to the DMA-hop norm path
XQUAD_NORM = True


def _rope_tables():
    """Full 128-partition rope tables (inlined; avoids SBUF broadcasts)."""
    inv = 1.0 / (ROPE_THETA ** (np.arange(0, DH, 2, dtype=np.float64) / DH))
    f = np.arange(T, dtype=np.float64)[:, None] * inv[None, :]  # [T, 32]
    cos = np.cos(f).T.astype(ml_dtypes.bfloat16)  # [32, T]
    sin = np.sin(f).T.astype(ml_dtypes.bfloat16)
    cs1 = np.ascontiguousarray(np.tile(cos, (4, 1)))  # [128, T]
    cs2 = np.ascontiguousarray(
        np.concatenate([sin, -sin, sin, -sin], axis=0)
    )  # [128, T]
    return cs1, cs2


def _build():
    nc = bacc.Bacc("TRN2", target_bir_lowering=False, debug=False, num_devices=NCORES)
    # host pre-arranged layouts (see kernel() below):
    #   xp:  [TC*128, DC*TCH]  chunk tci rows tci*128..: [p][dc*TCH+t']
    #   wq/wk/wv: [128, DC*HPC*DH]   [p][dc*256+n]
    #   wo:  [128, 2*D]              [p][p2*D+n]
    xp_d = nc.dram_tensor("xp", [TC * 128, DC * TCH], BF16, kind="ExternalInput")
    wq_d = nc.dram_tensor("wq", [128, DC * HPC * DH], BF16, kind="ExternalInput")
    wk_d = nc.dram_tensor("wk", [128, DC * HPC * DH], BF16, kind="ExternalInput")
    wv_d = nc.dram_tensor("wv", [128, DC * HPC * DH], BF16, kind="ExternalInput")
    wo_d = nc.dram_tensor("wo", [128, 2 * D], BF16, kind="ExternalInput")
    y_d = nc.dram_tensor("y", [T, D], BF16, kind="ExternalOutput")

    cs1_np, cs2_np = _rope_tables()
    cs1_d = nc.inline_tensor(cs1_np, "cs1")  # [128, T]
    cs2_d = nc.inline_tensor(cs2_np, "cs2")  # [128, T]

    # causal masks (keep = 1.0): maskt for the kt0 diagonal 128-block,
    # maskd = [zeros(128) | tri(128)] for the extended kt1 diagonal block
    maskt_np = (np.arange(128)[None, :] >= np.arange(128)[:, None])
    maskd_np = (np.arange(256)[None, :] >= np.arange(128)[:, None] + 128)
    maskt_d = nc.inline_tensor(
        np.ascontiguousarray(maskt_np.astype(ml_dtypes.bfloat16)), "maskt"
    )
    maskd_d = nc.inline_tensor(
        np.ascontiguousarray(maskd_np.astype(ml_dtypes.bfloat16)), "maskd"
    )
    # den-broadcast selector rows (both on partition 0 so each K=1 matmul's
    # lhsT is partition-aligned): bcps partitions 0-63 <- denr par1 (odd
    # head), partitions 64-127 <- denr par0 (even head); matches par-swapped ot
    sel_np = np.zeros((1, 2, 128), dtype=np.float32)
    sel_np[0, 1, 0:64] = 1.0
    sel_np[0, 0, 64:128] = 1.0
    sel_d = nc.inline_tensor(sel_np, "selc")

    EXP = mybir.ActivationFunctionType.Exp

    import contextlib
    with tile.TileContext(nc) as tc:
        with (
            contextlib.ExitStack() as _ctx,
            tc.tile_pool(name="sb", bufs=1) as sb,
            tc.tile_pool(name="ropep", bufs=2) as ropep,
            tc.tile_pool(name="ptp", bufs=5) as ptp,
            tc.tile_pool(name="orawp", bufs=2) as orawp,
            tc.tile_pool(name="miscp", bufs=2) as miscp,
            tc.tile_pool(name="ysbp", bufs=3) as ysbp,
        ):
            wq = sb.tile([128, DC, HPC * DH], BF16)
            wk = sb.tile([128, DC, HPC * DH], BF16)
            wv = sb.tile([128, DC, HPC * DH], BF16)
            wo = sb.tile([128, 2, D], BF16)
            cs1 = sb.tile([128, T], BF16)
            cs2 = sb.tile([128, T], BF16)
            qt = [sb.tile([128, T], BF16, tag=f"qt{p}", name=f"qt{p}") for p in range(2)]
            ktt = [sb.tile([128, T], BF16, tag=f"kt{p}", name=f"kt{p}") for p in range(2)]
            maskt = sb.tile([128, 128], BF16, name="maskt")
            maskd = sb.tile([128, 256], BF16, name="maskd")
            vaug = sb.tile([128, NKT // 2, 2, HPC, DH + 1], BF16, name="vaug")
            ot = [sb.tile([128, T], BF16, tag=f"ot{p}", name=f"ot{p}") for p in range(2)]
            sel = sb.tile([1, 2, 128], F32, name="sel")
            xts = [
                sb.tile([128, DC, TCH], BF16, tag=f"xt{tci}", name=f"xt{tci}")
                for tci in range(TC)
            ]

            # ---------------- startup DMAs ----------------
            # sync ring: chunk-0 x first (smallest latency to first proj MM)
            # sync ring carries only chunk-0 x and the rope swap DMAs, so the
            # first attention unit's rope chain is never queued behind bulk
            # loads. None of these issue instructions waits on anything, so no
            # ring engine ever stalls at its queue head during startup.
            # ring plan: the q-rope swaps ride sync, the k-rope swaps ride
            # gpsimd, so the two chains overlap and neither queues behind a
            # bulk preload. Scalar (which never issues DMAs after startup)
            # carries everything bulky.
            xp_r = xp_d.ap().rearrange("(c p) (d t) -> p c d t", p=128, d=DC)
            nc.sync.dma_start(xts[0][:], xp_r[:, 0])
            nc.sync.dma_start(maskt[:], maskt_d.ap())
            nc.sync.dma_start(maskd[:], maskd_d.ap())
            nc.scalar.dma_start(wq[:], wq_d.ap().rearrange("p (c n) -> p c n", c=DC))
            nc.scalar.dma_start(wk[:], wk_d.ap().rearrange("p (c n) -> p c n", c=DC))
            nc.scalar.dma_start(wv[:], wv_d.ap().rearrange("p (c n) -> p c n", c=DC))
            nc.scalar.dma_start(sel[:], sel_d.ap())
            for tci in range(1, TC):
                nc.scalar.dma_start(xts[tci][:], xp_r[:, tci])
            nc.scalar.dma_start(wo[:], wo_d.ap().rearrange("p (a n) -> p a n", a=2))
            nc.gpsimd.dma_start(cs1[:], cs1_d.ap())
            nc.gpsimd.dma_start(cs2[:], cs2_d.ap())

            nc.vector.memset(vaug[:, :, :, :, DH : DH + 1], 1.0)

            # PSUM: pjps(2) + sps(2 tags x 2 banks) + ops(2 tags x 1) = 8 banks
            pjps = _ctx.enter_context(tc.tile_pool(name="pjps", bufs=2, space="PSUM"))
            sps = _ctx.enter_context(tc.tile_pool(name="sps", bufs=1, space="PSUM"))
            ops = _ctx.enter_context(tc.tile_pool(name="ops", bufs=1, space="PSUM"))

            # PE warm-up: covers the startup DMA latency; ramps the PE p-state
            warm = sb.tile([128, TCH], BF16, name="warm")
            nc.vector.memset(warm, 0.0)
            wps = pjps.tile([128, TCH], F32, tag="pj", name="wps")
            for wi in range(12):
                nc.tensor.matmul(
                    wps[:], warm[:, 0:128], warm[:],
                    start=(wi == 0), stop=(wi == 11),
                )

            # ---------------- projection pieces ----------------
            def gen_proj_pieces(tci):
                """8 pieces: [q p0, q p1, k p0, k p1, v0..v3] (each ~8 MMs)."""
                tsl = slice(tci * TCH, (tci + 1) * TCH)
                xt = xts[tci]

                def qk_piece(w_sb, dest, p, pi):
                    is_q = dest is qt

                    def run():
                        ps = pjps.tile([128, TCH], F32, tag="pj", name=f"pj{tci}_{pi}")
                        for dc in range(DC):
                            nc.tensor.matmul(
                                ps[:],
                                w_sb[:, dc, p * 128 : (p + 1) * 128],
                                xt[:, dc, :],
                                start=(dc == 0),
                                stop=(dc == DC - 1),
                            )
                        # rope reads proj PSUM directly
                        t1 = ropep.tile([128, TCH], BF16, tag="t1", name=f"t1_{tci}_{pi}")
                        t2 = ropep.tile([128, TCH], BF16, tag="t2", name=f"t2_{tci}_{pi}")
                        swt = ropep.tile([128, TCH], BF16, tag="swt", name=f"sw{tci}_{pi}")
                        nc.vector.tensor_mul(t1[:], ps[:], cs1[:, tsl])
                        nc.vector.tensor_mul(t2[:], ps[:], cs2[:, tsl])
                        swring = nc.sync if is_q else nc.gpsimd
                        for s in range(4):
                            swring.dma_start(
                                swt[s * 32 : (s + 1) * 32, :],
                                t2[(s ^ 1) * 32 : ((s ^ 1) + 1) * 32, :],
                            )
                        nc.vector.tensor_add(dest[p][:, tsl], t1[:], swt[:])
                    return run

                def v_piece(tt):
                    def run():
                        gt = tci * 4 + tt
                        ps = pjps.tile([128, TCH], F32, tag="pj", name=f"pjv{gt}")
                        for dc in range(DC):
                            nc.tensor.matmul(
                                ps[:, : HPC * DH],
                                xt[:, dc, tt * 128 : (tt + 1) * 128],
                                wv[:, dc, :],
                                start=(dc == 0),
                                stop=(dc == DC - 1),
                            )
                        nc.vector.tensor_copy(
                            vaug[:, gt // 2, gt % 2, :, 0:DH],
                            ps[:, : HPC * DH].rearrange("p (h d) -> p h d", h=HPC),
                        )
                    return run

                pieces = []
                pi = 0
                for w_sb, dest in ((wq, qt), (wk, ktt)):
                    for p in range(2):
                        pieces.append(qk_piece(w_sb, dest, p, pi))
                        pi += 1
                for tt in range(4):
                    pieces.append(v_piece(tt))
                return pieces

            # ---------------- out-projection pieces ----------------
            YRINGS = [nc.sync, nc.gpsimd, nc.sync, nc.gpsimd]

            def gen_outproj_pieces(qc):
                def piece(tt):
                    def run():
                        gtt = qc * 4 + tt
                        ysb = ysbp.tile([128, 2, TCH], BF16, tag="ysb", name=f"ys{gtt}")
                        for ni in range(2):
                            ypsum = pjps.tile(
                                [128, TCH], F32, tag="pj", name=f"y{gtt}_{ni}"
                            )
                            for p2 in range(2):
                                nc.tensor.matmul(
                                    ypsum[:],
                                    ot[p2][:, gtt * 128 : (gtt + 1) * 128],
                                    wo[:, p2, ni * TCH : (ni + 1) * TCH],
                                    start=(p2 == 0),
                                    stop=(p2 == 1),
                                )
                            nc.vector.tensor_copy(ysb[:, ni, :], ypsum[:])
                        YRINGS[tt].dma_start(
                            y_d.ap()[gtt * 128 : (gtt + 1) * 128, :],
                            ysb.rearrange("p a b -> p (a b)"),
                        )
                    return run
                return [piece(tt) for tt in range(4)]

            # ---------------- attention: head-pair units ----------------
            pts = {}     # (qc,hp) -> {g: [pt_par0, pt_par1]}
            opsums = {}  # (qc,hp) -> [psum_par0, psum_par1]

            def emit_s_group(qc, hp, g):
                """Row-tiled S pair + exp + mask for k-tile pair (2g, 2g+1)."""
                kt0 = 2 * g
                off0 = max(0, 128 * kt0 - TCH * qc)
                qsl = slice(qc * TCH, (qc + 1) * TCH)
                spts, ptts = [], []
                for par in (0, 1):
                    spt = sps.tile(
                        [128, 2, TCH], F32, tag=f"s{par}", name=f"s{qc}_{hp}_{g}_{par}"
                    )
                    pt = ptp.tile(
                        [128, 2, TCH], BF16, tag=f"pt{par}", name=f"p{qc}_{hp}_{g}_{par}"
                    )
                    spts.append(spt)
                    ptts.append(pt)
                pts[(qc, hp)][g] = ptts
                for j in (0, 1):
                    kt = kt0 + j
                    for par in (0, 1):
                        nc.tensor.matmul(
                            spts[par][:, j, off0:],
                            ktt[hp][par * 64 : (par + 1) * 64, kt * 128 : (kt + 1) * 128],
                            qt[hp][par * 64 : (par + 1) * 64, qsl][:, off0:],
                            start=True,
                            stop=True,
                        )
                for par in (0, 1):
                    if off0 == 0:
                        nc.scalar.activation(
                            ptts[par].rearrange("p a b -> p (a b)"),
                            spts[par].rearrange("p a b -> p (a b)"),
                            EXP, bias=0.0, scale=0.125,
                        )
                    else:
                        nc.scalar.activation(
                            ptts[par][:, :, off0:], spts[par][:, :, off0:],
                            EXP, bias=0.0, scale=0.125,
                        )
                if kt0 >= 4 * qc:  # diagonal pair
                    for par in (0, 1):
                        nc.vector.tensor_mul(
                            ptts[par][:, 0, off0 : off0 + 128],
                            ptts[par][:, 0, off0 : off0 + 128],
                            maskt[:],
                        )
                        nc.vector.tensor_mul(
                            ptts[par][:, 1, off0 : off0 + 256],
                            ptts[par][:, 1, off0 : off0 + 256],
                            maskd[:],
                        )

            def emit_av_group(qc, hp, g):
                nkt = 4 * qc + 4
                kt0 = 2 * g
                ptts = pts[(qc, hp)].pop(g)
                for par in (0, 1):
                    if g == 0:
                        opsums[(qc, hp)][par] = ops.tile(
                            [128, TCH], F32, tag=f"o{par}", name=f"o{qc}_{hp}_{par}"
                        )
                    h = 2 * hp + par
                    for j in (0, 1):
                        kt = kt0 + j
                        off = max(0, 128 * kt - TCH * qc)
                        nc.tensor.matmul(
                            opsums[(qc, hp)][par][0 : DH + 1, off:],
                            vaug[:, g, j, h, 0 : DH + 1],
                            ptts[par][:, j, off:],
                            start=(kt == 0),
                            stop=(kt == nkt - 1),
                        )

            def emit_evac_norm(qc, hp):
                """Evacuate both heads' AV psums, reciprocal of dens, broadcast
                via a tiny matmul, scale into ot. ot layout par-swapped:
                partitions 0-63 = odd head, 64-127 = even head."""
                qsl = slice(qc * TCH, (qc + 1) * TCH)
                oraw2 = []
                for par in (0, 1):
                    oraw = orawp.tile(
                        [128, TCH], F32, tag=f"or{par}", name=f"or{qc}_{hp}_{par}"
                    )
                    oraw2.append(oraw)
                    nc.vector.tensor_copy(
                        oraw[0 : DH + 1, :], opsums[(qc, hp)][par][0 : DH + 1, :]
                    )
                denr = miscp.tile([1, 2, TCH], F32, tag="denr", name=f"dr{qc}_{hp}")
                denp = miscp.tile([1, 2, TCH], F32, tag="denp", name=f"dp{qc}_{hp}")
                for par in (0, 1):
                    nc.sync.dma_start(
                        denp[:, par, :], oraw2[par][DH : DH + 1, :]
                    )
                nc.vector.reciprocal_approx_fast(
                    out=denr.rearrange("p a b -> p (a b)"),
                    in_=denp.rearrange("p a b -> p (a b)"),
                )
                bcps = ops.tile([128, TCH], F32, tag="o0", name=f"bc{qc}_{hp}")
                nc.tensor.matmul(bcps[:], sel[:, 1, :], denr[:, 1, :], start=True, stop=False)
                nc.tensor.matmul(bcps[:], sel[:, 0, :], denr[:, 0, :], start=False, stop=True)
                nc.vector.tensor_mul(
                    ot[hp][0:64, qsl], oraw2[1][0:64, :], bcps[0:64, :]
                )
                if XQUAD_NORM:
                    nc.vector.tensor_mul(
                        ot[hp][64:128, qsl], oraw2[0][0:64, :], bcps[64:128, :]
                    )
                else:
                    tmpo = miscp.tile([64, TCH], BF16, tag="tmpo", name=f"tp{qc}_{hp}")
                    nc.vector.tensor_mul(
                        tmpo[:], oraw2[0][0:64, :], bcps[64:128, :]
                    )
                    nc.sync.dma_start(ot[hp][64:128, qsl], tmpo[:])

            # ---------------- emission schedule ----------------
            # proj(0) upfront; the two pieces the first attention unit needs
            # (q p0, k p0) go first
            p0 = gen_proj_pieces(0)
            for f in [p0[0], p0[2], p0[1], p0[3]] + p0[4:]:
                f()

            p1 = gen_proj_pieces(1)
            p2 = gen_proj_pieces(2)
            p3 = gen_proj_pieces(3)

            op0 = gen_outproj_pieces(0)
            op1 = gen_outproj_pieces(1)
            op2 = gen_outproj_pieces(2)

            units = [(qc, hp) for qc in range(TC) for hp in range(2)]
            # fillers per unit (consumed one per S-group slot; leftovers run
            # at the unit boundary). Invocation order must respect the norms
            # each outproj chunk reads.
            unit_fillers = {
                0: [p1[0], p1[2], p1[1], p1[3]],      # q1/k1 (2 slots + spill)
                1: p1[4:8],                           # v(1)
                2: [p2[0], p2[2], p2[1], p2[3]],
                3: op0 + p2[4:8],                     # outproj(0), v(2)
                4: [p3[0], p3[2], p3[1], p3[3]],
                5: op1,
                6: p3[4:8],                           # v(3)
                7: op2,
            }

            # flat S-slot stream with a fixed AV lag: the AV matmuls for
            # S slot i run at slot i+LAG, so the final unit's AV doesn't
            # bunch up in the drain
            LAG = 3
            slots = []
            for ui, (qc, hp) in enumerate(units):
                for g in range((4 * qc + 4) // 2):
                    slots.append((ui, qc, hp, g))

            def do_av_slot(j):
                ui, qc, hp, g = slots[j]
                emit_av_group(qc, hp, g)
                if g == (4 * qc + 4) // 2 - 1:  # unit's last AV group
                    emit_evac_norm(qc, hp)

            fill = []
            for i, (ui, qc, hp, g) in enumerate(slots):
                if g == 0:
                    fill = list(unit_fillers.get(ui, []))
                    pts[(qc, hp)] = {}
                    opsums[(qc, hp)] = [None, None]
                if fill:
                    fill.pop(0)()
                if i >= LAG:
                    do_av_slot(i - LAG)
                emit_s_group(qc, hp, g)
                if g == (4 * qc + 4) // 2 - 1:
                    for f in fill:
                        f()
                    fill = []

            # drain: the last LAG AV slots, final norm, outproj of last chunk
            for j in range(len(slots) - LAG, len(slots)):
                do_av_slot(j)
            for f in gen_outproj_pieces(3):
                f()
    nc.compile()
    return nc


_NC_CACHE = []


def _get_nc():
    if not _NC_CACHE:
        _NC_CACHE.append(_build())
    return _NC_CACHE[0]


_LAST_RESULTS = []  # stashed BassKernelResults for test harness introspection


def _wo_rows_parswap(Wout_rows):
    """Reorder the 256 Wout rows so each 128-row pair block is [odd-head 64 | even-head 64]."""
    out = np.empty_like(Wout_rows)
    for hp in range(2):
        blk = Wout_rows[hp * 128 : (hp + 1) * 128]
        out[hp * 128 : hp * 128 + 64] = blk[64:128]
        out[hp * 128 + 64 : (hp + 1) * 128] = blk[0:64]
    return out


def kernel(x, Wqkv, Wout, _trace=False, **_trace_kwargs):
    x = np.asarray(x, dtype=np.float32)
    Wqkv = np.asarray(Wqkv, dtype=np.float32)
    Wout = np.asarray(Wout, dtype=np.float32)

    nc = _get_nc()
    bf = ml_dtypes.bfloat16
    in_maps = []
    for c in range(NCORES):
        b, g = divmod(c, HPC)
        cols = slice(g * HPC * DH, (g + 1) * HPC * DH)
        rows = slice(g * HPC * DH, (g + 1) * HPC * DH)
        xT = x[b].T.astype(bf)  # [D, T]
        xp = np.ascontiguousarray(
            xT.reshape(DC, 128, TC, TCH).transpose(2, 1, 0, 3).reshape(TC * 128, DC * TCH)
        )

        def wprep(w):  # [D, 256] -> [128, DC*256]
            return np.ascontiguousarray(
                w.astype(bf).reshape(DC, 128, HPC * DH).transpose(1, 0, 2).reshape(128, -1)
            )

        wo_ = np.ascontiguousarray(
            _wo_rows_parswap(Wout[rows, :]).astype(bf)
            .reshape(2, 128, D).transpose(1, 0, 2).reshape(128, 2 * D)
        )
        in_maps.append(
            {
                "xp": xp,
                "wq": wprep(Wqkv[:, 0:D][:, cols]),
                "wk": wprep(Wqkv[:, D : 2 * D][:, cols]),
                "wv": wprep(Wqkv[:, 2 * D : 3 * D][:, cols]),
                "wo": wo_,
            }
        )

    res = run_bass_kernel_spmd(
        nc, in_maps, core_ids=list(range(NCORES)), trace=_trace, **_trace_kwargs
    )
    _LAST_RESULTS.clear()
    _LAST_RESULTS.append(res)

    out = np.zeros((B, T, D), dtype=np.float32)
    for c in range(NCORES):
        b = c // HPC
        out[b] += res.results[c]["y"].astype(np.float32)
    return out
